# revision 24
# baseline (speedup 1.0000x reference)
"""GNN message-passing (gated GCN style) on 8 Trainium2 NeuronCores.

v2 strategy (edge-parallel, dst-sorted shards):
- Host sorts edges by dst, splits into 8 shards snapped to node-run
  boundaries; each device owns a contiguous node range so segment_max is
  local. Nodes sorted by in-degree; each node's run padded to per-tile
  slot count S (pow2, >=4), so segment_max is a fixed-window reduce_max.
- Groups of 4 chunks (512 edges) are tile-aligned: one matmul/activation/
  reduce instruction covers 512 edges.
- Per layer each device computes its node-table slice ([hV|hC] for l0,
  hV1, hW0i), AllGathers it in fp32; per-edge src gathers are 512B-row
  indirect DMAs (two per chunk for l0 via element_offset, one otherwise).
- Embedding biases ride a mask row in the input streams so padded slots
  stay exactly zero -> BatchNorm statistics need no masking; stats are
  accumulated via accum_out on DVE/Act ops and combined with one small
  AllReduce per layer.
- dst-side terms (B0h[dst], W0j h2[dst]) are expanded from node-major
  SBUF tables with a degree-masked kron matmul streamed per tile; the
  src-side C contribution transpose-accumulates into the same PSUM group.
"""

import numpy as np

NC = 8
D = 128


# ---------------------------------------------------------------------------
# host-side planning
# ---------------------------------------------------------------------------


def _next_pow2(x):
    p = 1
    while p < x:
        p *= 2
    return p


def _plan(src, dst, N):
    E = src.shape[0]
    order = np.argsort(dst, kind="stable")
    dsts = dst[order]
    srcs = src[order]

    bounds = [0]
    for r in range(1, NC):
        t = (E * r) // NC
        b = int(np.searchsorted(dsts, dsts[t], side="left"))
        bounds.append(max(b, bounds[-1]))
    bounds.append(E)

    lo = np.zeros(NC, np.int64)
    for d in range(1, NC):
        lo[d] = int(dsts[bounds[d]]) if bounds[d] < E else N
    hi = np.empty(NC, np.int64)
    hi[:-1] = lo[1:]
    hi[-1] = N

    n_r = [int(hi[d] - lo[d]) for d in range(NC)]
    NODE_CAP = 128 * int(np.ceil((max(n_r) + 2) / 128))
    T = NODE_CAP // 128

    shards = []
    for d in range(NC):
        sl = slice(bounds[d], bounds[d + 1])
        dl = dsts[sl] - lo[d]
        cnt = np.bincount(dl, minlength=n_r[d]) if n_r[d] > 0 else np.zeros(0, int)
        starts = np.concatenate([[0], np.cumsum(cnt)])
        perm = np.argsort(-cnt, kind="stable") if n_r[d] > 0 else np.zeros(0, int)
        ipos = np.empty(n_r[d], np.int64)
        ipos[perm] = np.arange(n_r[d])
        shards.append(
            dict(sl=sl, dl=dl, cnt=cnt, starts=starts, perm=perm, ipos=ipos, d=d)
        )

    # shared per-tile slot counts, min 4 so 4-chunk groups are tile-aligned
    S_list = []
    for t in range(T):
        mx = 1
        for sh in shards:
            p = sh["perm"][t * 128 : (t + 1) * 128]
            if len(p):
                c = sh["cnt"][p]
                if len(c):
                    mx = max(mx, int(c.max()))
        S_list.append(min(max(_next_pow2(mx), 4), 128))

    E_PAD = 128 * int(np.sum(S_list))
    C_E = E_PAD // 128
    chunks = []  # (tile, ci, S)
    for t in range(T):
        for ci in range(S_list[t]):
            chunks.append((t, ci, S_list[t]))
    groups = []  # (tile, j, S, c0); always within one tile (S >= 4, pow2)
    c = 0
    for t in range(T):
        S = S_list[t]
        for j in range(S // 4):
            groups.append((t, j, S, c))
            c += 4
    assert c == C_E

    return dict(
        E=E,
        N=N,
        order=order,
        srcs=srcs,
        bounds=bounds,
        lo=np.asarray(lo),
        hi=np.asarray(hi),
        n_r=n_r,
        NODE_CAP=NODE_CAP,
        T=T,
        S_list=S_list,
        E_PAD=E_PAD,
        C_E=C_E,
        chunks=chunks,
        groups=groups,
        shards=shards,
    )


def _kron_mask_tile(plan, d, t):
    """[128, 512] degree-masked kron block for tile t of shard d."""
    S = plan["S_list"][t]
    G = 128 // S
    sh = plan["shards"][d]
    perm = sh["perm"]
    cnt = sh["cnt"]
    out = np.zeros((128, 512), np.float32)
    for p in range(128):
        pos = t * 128 + p
        if pos >= len(perm):
            continue
        deg = int(min(cnt[perm[pos]], S))
        if deg == 0:
            continue
        k = (p // G) % 4
        g = p % G
        out[p, k * 128 + g * S : k * 128 + g * S + deg] = 1.0
    return out


def _per_core_arrays(plan, d, h, e, bf16):
    sh = plan["shards"][d]
    NODE_CAP, T = plan["NODE_CAP"], plan["T"]
    S_list = plan["S_list"]
    E_PAD, C_E = plan["E_PAD"], plan["C_E"]
    n_r = plan["n_r"][d]
    guard_row = d * NODE_CAP + (NODE_CAP - 1)

    e_sh = e[plan["order"]][sh["sl"]]  # [E_s, F_E]
    src_sh = plan["srcs"][sh["sl"]]
    orig_sh = np.arange(plan["E"])[plan["order"]][sh["sl"]]

    F_E = e.shape[1]
    e0_pad = np.zeros((E_PAD, F_E), np.float32)
    hsrc_pad = np.zeros((E_PAD, h.shape[1]), np.float32)
    maskf = np.zeros(E_PAD, np.float32)
    srcrow = np.full(E_PAD, guard_row, np.int64)
    origid = np.full(E_PAD, -1, np.int64)

    base = 0
    perm = sh["perm"]
    cnt = sh["cnt"]
    starts = sh["starts"]
    for t in range(T):
        S = S_list[t]
        pn = perm[t * 128 : (t + 1) * 128]
        im = np.full((128, S), -1, np.int64)
        for i, n in enumerate(pn):
            dg = int(cnt[n])
            k = min(dg, S)
            if k:
                im[i, :k] = np.arange(starts[n], starts[n] + k)
        flat = im.reshape(-1)
        real = flat >= 0
        fr = flat[real]
        blk = slice(base, base + 128 * S)
        e0_blk = np.zeros((128 * S, F_E), np.float32)
        e0_blk[real] = e_sh[fr]
        e0_pad[blk] = e0_blk
        mk = np.zeros(128 * S, np.float32)
        mk[real] = 1.0
        maskf[blk] = mk
        sr = np.full(128 * S, guard_row, np.int64)
        g = src_sh[fr]
        r = np.clip(np.searchsorted(plan["lo"], g, side="right") - 1, 0, NC - 1)
        loc = g - plan["lo"][r]
        pp_ = np.empty(len(g), np.int64)
        for rr in np.unique(r):
            m = r == rr
            pp_[m] = plan["shards"][rr]["ipos"][loc[m]]
        sr[real] = r * NODE_CAP + pp_
        srcrow[blk] = sr
        oi = np.full(128 * S, -1, np.int64)
        oi[real] = orig_sh[fr]
        origid[blk] = oi
        hsrc_blk = np.zeros((128 * S, h.shape[1]), np.float32)
        hsrc_blk[real] = h[g]
        hsrc_pad[blk] = hsrc_blk
        base += 128 * S

    # feat-major streams with a trailing mask row
    e0T = np.concatenate([e0_pad.T, maskf[None, :]], axis=0)  # [F_E+1, E_PAD]
    srcidx = srcrow.reshape(C_E, 128).T.astype(np.int32).copy()

    # raw src-node features per edge slot (layer-0 msg/C are linear in h):
    # rows 0..F_N-1 = h_raw[src], row F_N = mask, row F_N+1 = 1-mask
    hsT = np.concatenate(
        [hsrc_pad.T, maskf[None, :], (1.0 - maskf)[None, :]], axis=0
    )

    F_N = h.shape[1]
    h0p = np.zeros((NODE_CAP, F_N), np.float32)
    hl = h[plan["lo"][d] : plan["hi"][d]]
    h0p[: len(perm)] = hl[perm]
    nodemask = np.zeros(NODE_CAP, np.float32)
    nodemask[:n_r] = 1.0
    h0T = np.concatenate([h0p.T, nodemask[None, :]], axis=0)  # [F_N+1, NODE_CAP]

    kron_m = np.concatenate(
        [_kron_mask_tile(plan, d, t) for t in range(T)], axis=0
    )  # [T*128, 512]
    nmb = np.broadcast_to(nodemask[None, :], (128, NODE_CAP)).copy()

    return dict(
        h0T=bf16(h0T),
        e0T=bf16(e0T),
        hsT=bf16(hsT),
        srcidx=srcidx,
        kron_m=bf16(kron_m),
        nmb=bf16(nmb),
        origid=origid,
    )


# ---------------------------------------------------------------------------
# device program
# ---------------------------------------------------------------------------


def _build_program(plan, stop_after="full", ablate=None):
    import concourse.mybir as mybir
    import concourse.tile as tile
    from concourse import bacc
    from concourse.bass import IndirectOffsetOnAxis

    F32 = mybir.dt.float32
    TDT = mybir.dt.bfloat16
    RODT = mybir.dt.bfloat16
    I32 = mybir.dt.int32
    AF = mybir.ActivationFunctionType
    OP = mybir.AluOpType
    AX = mybir.AxisListType

    NODE_CAP, T = plan["NODE_CAP"], plan["T"]
    E_PAD, C_E = plan["E_PAD"], plan["C_E"]
    groups = plan["groups"]
    NG = len(groups)
    N, E = plan["N"], plan["E"]
    F_N, F_E = plan["F_N"], plan["F_E"]
    TN = T * 128
    NSL = (T * 128 + 511) // 512  # 512-wide node slabs
    EPS = 1e-5

    _phases = ["embed", "bound0", "epass0", "layer0", "layer1", "full"]
    lvl = _phases.index(stop_after)

    nc = bacc.Bacc("TRN2", target_bir_lowering=False, debug=False, num_devices=NC)

    def din(name, shape, dt=F32):
        return nc.dram_tensor(name, shape, dt, kind="ExternalInput")

    # per-core inputs
    h0T = din("h0T", [F_N + 1, NODE_CAP], TDT)
    e0T = din("e0T", [F_E + 1, E_PAD], TDT)
    hsT_d = din("hsT", [F_N + 2, E_PAD], TDT)
    srcidx = din("srcidx", [128, C_E], I32)
    kron_md = din("kron_m", [TN, 512], TDT)
    nmb_d = din("nmb", [128, NODE_CAP], TDT)
    # shared weights
    ident_d = din("ident", [128, 128])
    gne_d = din("gne", [1, 256])
    ew9_d = din("ew9", [F_E + 1, 128], TDT)
    eA9_d = din("eA9", [F_E + 1, 128], TDT)
    hw17_d = din("hw17", [F_N + 1, 128], TDT)
    B0_d = din("B0", [128, 128], TDT)
    WnV18_d = din("WnV18", [F_N + 2, 128], TDT)
    WnC18_d = din("WnC18", [F_N + 2, 128], TDT)
    V1_d = din("V1", [128, 128], TDT)
    U0_d = din("U0", [128, 128], TDT)
    U1_d = din("U1", [128, 128], TDT)
    WBC_d = din("WBC", [128, 256], TDT)
    W0a_d = din("W0a", [128, 128])
    W0b_col_d = din("W0b_col", [128, 1])
    Wk0_d = din("Wk0", [128, 128], RODT)
    Wk1_d = din("Wk1", [128, 128], RODT)
    Wkb0_d = din("Wkb0", [128, 1])
    Wkb1_d = din("Wkb1", [128, 1])
    Wf_d = din("Wf", [128, 1], RODT)
    wfb_d = din("wfb", [1, 1])

    y_out = nc.dram_tensor("y", [1, E_PAD], F32, kind="ExternalOutput")

    rg = [list(range(NC))]

    with tile.TileContext(nc) as tc:
        with (
            tc.tile_pool(name="const", bufs=1) as cp,
            tc.tile_pool(name="pers", bufs=1) as pp,
            tc.tile_pool(name="st", bufs=1) as stp,
            tc.tile_pool(name="s", bufs=2) as sp,
            tc.tile_pool(name="ps", bufs=2, space="PSUM") as ps,
            tc.tile_pool(name="dram", bufs=1, space="DRAM") as dp,
        ):
            def cload(dram_t, shape, dt=F32, name=None):
                t = cp.tile(shape, dt, name=name or dram_t.name + "_sb")
                nc.sync.dma_start(out=t[:], in_=dram_t[:])
                return t

            ident = cload(ident_d, [128, 128])
            gne = cload(gne_d, [1, 256])
            ew9 = cload(ew9_d, [F_E + 1, 128], TDT)
            eA9 = cload(eA9_d, [F_E + 1, 128], TDT)
            hw17 = cload(hw17_d, [F_N + 1, 128], TDT)
            B0 = cload(B0_d, [128, 128], TDT)
            WnV18 = cload(WnV18_d, [F_N + 2, 128], TDT)
            WnC18 = cload(WnC18_d, [F_N + 2, 128], TDT)
            V1 = cload(V1_d, [128, 128], TDT)
            U0 = cload(U0_d, [128, 128], TDT)
            U1 = cload(U1_d, [128, 128], TDT)
            WBC = cload(WBC_d, [128, 256], TDT)
            W0a = cload(W0a_d, [128, 128])
            W0bc = cload(W0b_col_d, [128, 1])
            Wk0 = cload(Wk0_d, [128, 128], RODT)
            Wk1 = cload(Wk1_d, [128, 128], RODT)
            Wkb0 = cload(Wkb0_d, [128, 1])
            Wkb1 = cload(Wkb1_d, [128, 1])
            Wf = cload(Wf_d, [128, 1], RODT)
            wfb = cload(wfb_d, [1, 1])
            srci = cload(srcidx, [128, C_E], I32)
            nm_all = cload(nmb_d, [128, NODE_CAP], TDT)
            eps_col = cp.tile([128, 1], F32, name="eps_col")
            nc.gpsimd.memset(eps_col[:], EPS)

            # dram buffers
            z_buf = dp.tile([128, E_PAD], TDT, name="z_buf")
            hb_buf = dp.tile([NODE_CAP, 128], TDT, name="hb_buf")
            cc_hin = {
                l: dp.tile([NODE_CAP, 128], F32, name=f"cc_hin{l}")
                for l in (1, 2)
            }
            cc_hout = {
                l: dp.tile(
                    [NC * NODE_CAP, 128],
                    F32,
                    name=f"cc_hout{l}",
                    addr_space="Shared",
                )
                for l in (1, 2)
            }
            cc_st_in = [
                dp.tile([128, 4 if l == 0 else 2], F32, name=f"cc_st_in{l}")
                for l in range(2)
            ]
            cc_st_out = [
                dp.tile(
                    [128, 4 if l == 0 else 2], F32,
                    name=f"cc_st_out{l}", addr_space="Shared",
                )
                for l in range(2)
            ]
            cc_moy_in = dp.tile([128, 1], F32, name="cc_moy_in")
            cc_moy_out = dp.tile([128, 1], F32, name="cc_moy_out", addr_space="Shared")

            # persistent sbuf
            hA = pp.tile([128, TN], TDT, name="hA")
            hB = pp.tile([128, TN], TDT, name="hB")
            hC = pp.tile([128, TN], TDT, name="hC")
            hU = pp.tile([128, TN], TDT, name="hU")
            agg = pp.tile([128, TN], TDT, name="agg")

            def slab_cols(s):
                c0 = s * 512
                return c0, min(512, TN - c0)

            # ================= embed h =================
            for s in range(NSL):
                c0, w = slab_cols(s)
                h0sl = sp.tile([F_N + 1, 512], TDT, tag="h0sl", bufs=2)
                nc.sync.dma_start(out=h0sl[:, :w], in_=h0T[:, c0 : c0 + w])
                ph = ps.tile([128, 512], F32, tag="pA")
                nc.tensor.matmul(
                    out=ph[:, :w], lhsT=hw17[:], rhs=h0sl[:, :w],
                    start=True, stop=True,
                )
                nc.scalar.activation(
                    out=hA[:, c0 : c0 + w], in_=ph[:, :w], func=AF.Copy
                )

            # ================= boundaries =================
            def boundary(l, hsrc):
                if l == 0:
                    rhs, wdt = B0, 128
                elif l == 1:
                    rhs, wdt = V1, 128
                else:
                    rhs, wdt = WBC, 256
                U = U0 if l == 0 else (U1 if l == 1 else None)
                for t in range(T):
                    pb = ps.tile([128, 512], F32, tag="pD")
                    nc.tensor.matmul(
                        out=pb[:, :wdt],
                        lhsT=hsrc[:, t * 128 : (t + 1) * 128],
                        rhs=rhs[:],
                        start=True, stop=True,
                    )
                    if l in (1, 2):
                        bsb = sp.tile([128, 128], F32, tag="bsb", bufs=4)
                        nc.scalar.activation(
                            out=bsb[:], in_=pb[:, 0:128], func=AF.Copy
                        )
                        nc.scalar.dma_start(
                            out=cc_hin[l][t * 128 : (t + 1) * 128, :],
                            in_=bsb[:],
                        )
                    if l in (0, 2):
                        hbst = sp.tile([128, 128], TDT, tag="hbst", bufs=4)
                        nc.scalar.activation(
                            out=hbst[:],
                            in_=pb[:, 0:128] if l == 0 else pb[:, 128:256],
                            func=AF.Copy,
                        )
                        nc.sync.dma_start(
                            out=hb_buf[t * 128 : (t + 1) * 128, :], in_=hbst[:]
                        )
                if U is not None:
                    for s in range(NSL):
                        c0, w = slab_cols(s)
                        pu = ps.tile([128, 512], F32, tag="pA")
                        nc.tensor.matmul(
                            out=pu[:, :w], lhsT=U[:], rhs=hsrc[:, c0 : c0 + w],
                            start=True, stop=True,
                        )
                        nc.scalar.activation(
                            out=hU[:, c0 : c0 + w], in_=pu[:, :w], func=AF.Copy
                        )
                if l in (1, 2):
                    gslice = gne[:, 0:128] if l == 1 else gne[:, 128:256]
                    nc.sync.dma_start(
                        out=cc_hin[l][NODE_CAP - 1 : NODE_CAP, :],
                        in_=gslice,
                    )
                    nc.gpsimd.collective_compute(
                        "AllGather", OP.bypass, replica_groups=rg,
                        ins=[cc_hin[l][:]], outs=[cc_hout[l][:]],
                    )

            if lvl >= 1:
                boundary(0, hA)

            # ================= e pass (layer 0) =================
            ssum_e = stp.tile([128, NG], F32, name="ssum_e")
            ssq_e = stp.tile([128, NG], F32, name="ssq_e")
            hsum = [stp.tile([128, NSL], F32, name=f"hsum{l}") for l in range(2)]
            hssq = [stp.tile([128, NSL], F32, name=f"hssq{l}") for l in range(2)]
            moys = stp.tile([128, NSL], F32, name="moys")

            if lvl >= 2:
                for gi, (t, j, S, c0) in enumerate(groups):
                    G = 128 // S
                    ec0 = c0 * 128
                    if gi % 4 == 0:
                        ew = min(2048, E_PAD - ec0)
                        e0sl = sp.tile([F_E + 1, 2048], TDT, tag="e0sl", bufs=3)
                        nc.sync.dma_start(
                            out=e0sl[:, :ew], in_=e0T[:, ec0 : ec0 + ew]
                        )
                        hssl = sp.tile([F_N + 2, 2048], TDT, tag="hssl", bufs=3)
                        nc.sync.dma_start(
                            out=hssl[:, :ew], in_=hsT_d[:, ec0 : ec0 + ew]
                        )
                        zstage = sp.tile([128, 2048], TDT, tag="zst", bufs=2)
                    o = (gi % 4) * 512
                    r0 = 4 * j * G
                    kront = sp.tile([128, 512], TDT, tag="kront", bufs=6)
                    nc.sync.dma_start(
                        out=kront[: 4 * G, :],
                        in_=kron_md[t * 128 + r0 : t * 128 + r0 + 4 * G, :],
                    )
                    band = sp.tile([128, 128], TDT, tag="band", bufs=6)
                    nc.sync.dma_start(
                        out=band[: 4 * G, :],
                        in_=hb_buf[t * 128 + r0 : t * 128 + r0 + 4 * G, :],
                    )
                    pw = ps.tile([128, 512], F32, tag="pA" if gi % 2 == 0 else "pD")
                    nc.tensor.matmul(
                        out=pw[:], lhsT=ew9[:], rhs=e0sl[:, o : o + 512],
                        start=True, stop=True,
                    )
                    wt = sp.tile([128, 512], F32, tag="wt", bufs=4)
                    nc.scalar.activation(out=wt[:], in_=pw[:], func=AF.Sigmoid)
                    pz = ps.tile([128, 512], F32, tag="pB")
                    nc.tensor.matmul(
                        out=pz[:], lhsT=eA9[:], rhs=e0sl[:, o : o + 512],
                        start=True, stop=False, skip_group_check=True,
                    )
                    nc.tensor.matmul(
                        out=pz[:], lhsT=WnC18[:], rhs=hssl[:, o : o + 512],
                        start=False, stop=False, skip_group_check=True,
                    )
                    nc.tensor.matmul(
                        out=pz[:],
                        lhsT=band[: 4 * G, :],
                        rhs=kront[: 4 * G, :],
                        start=False, stop=True, skip_group_check=True,
                    )
                    pvc = ps.tile([128, 512], F32, tag="pC")
                    nc.tensor.matmul(
                        out=pvc[:], lhsT=WnV18[:], rhs=hssl[:, o : o + 512],
                        start=True, stop=True,
                    )
                    msg = sp.tile([128, 512], TDT, tag="msg", bufs=3)
                    nc.vector.tensor_tensor(
                        out=msg[:], in0=pvc[:], in1=wt[:], op=OP.mult
                    )
                    nc.vector.tensor_reduce(
                        out=agg[:, t * 128 + r0 : t * 128 + r0 + 4 * G],
                        in_=msg[:].rearrange("p (g s) -> p g s", s=S),
                        op=OP.max, axis=AX.X,
                    )
                    nc.vector.tensor_scalar(
                        out=zstage[:, o : o + 512], in0=pz[:], scalar1=1.0,
                        scalar2=0.0, op0=OP.mult, op1=OP.add,
                        accum_out=ssum_e[:, gi : gi + 1],
                    )
                    sqs = sp.tile([128, 512], TDT, tag="sqs", bufs=2)
                    nc.scalar.activation(
                        out=sqs[:], in_=zstage[:, o : o + 512], func=AF.Square,
                        accum_out=ssq_e[:, gi : gi + 1],
                    )
                    if gi % 4 == 3 or gi == NG - 1:
                        sc0 = (gi // 4) * 2048
                        sw = min(2048, E_PAD - sc0)
                        nc.sync.dma_start(
                            out=z_buf[:, sc0 : sc0 + sw], in_=zstage[:, :sw]
                        )

            # ---- h-side helpers (defined regardless of lvl)
            def h_stats(l):
                for s in range(NSL):
                    c0, w = slab_cols(s)
                    ag = agg[:, c0 : c0 + w]
                    m01 = sp.tile([128, 512], TDT, tag="m01", bufs=2)
                    nc.vector.tensor_scalar(
                        out=m01[:, :w], in0=ag, scalar1=-1e20, scalar2=None,
                        op0=OP.is_gt,
                    )
                    nc.vector.tensor_tensor(
                        out=ag, in0=ag, in1=m01[:, :w], op=OP.mult
                    )
                    nc.vector.scalar_tensor_tensor(
                        out=ag, in0=ag, scalar=1.0, in1=hU[:, c0 : c0 + w],
                        op0=OP.mult, op1=OP.add,
                        accum_out=hsum[l][:, s : s + 1],
                    )
                    sqh = sp.tile([128, 512], TDT, tag="sqs", bufs=2)
                    nc.scalar.activation(
                        out=sqh[:, :w], in_=ag, func=AF.Square,
                        accum_out=hssq[l][:, s : s + 1],
                    )

            def bn_cols(sum_c, ssq_c, count, pref):
                mean = stp.tile([128, 1], F32, name=f"{pref}mean")
                nc.vector.tensor_scalar(
                    out=mean[:], in0=sum_c, scalar1=1.0 / count,
                    scalar2=None, op0=OP.mult,
                )
                msq = stp.tile([128, 1], F32, name=f"{pref}msq")
                nc.vector.tensor_scalar(
                    out=msq[:], in0=ssq_c, scalar1=1.0 / count,
                    scalar2=None, op0=OP.mult,
                )
                m2 = stp.tile([128, 1], F32, name=f"{pref}m2")
                nc.scalar.activation(out=m2[:], in_=mean[:], func=AF.Square)
                var = stp.tile([128, 1], F32, name=f"{pref}var")
                nc.vector.tensor_tensor(
                    out=var[:], in0=msq[:], in1=m2[:], op=OP.subtract
                )
                sd = stp.tile([128, 1], F32, name=f"{pref}sd")
                nc.scalar.activation(
                    out=sd[:], in_=var[:], func=AF.Sqrt, bias=eps_col[:]
                )
                rs = stp.tile([128, 1], F32, name=f"{pref}rs")
                nc.vector.reciprocal(out=rs[:], in_=sd[:])
                bb = stp.tile([128, 1], F32, name=f"{pref}bb")
                nc.vector.tensor_tensor(
                    out=bb[:], in0=mean[:], in1=rs[:], op=OP.mult
                )
                nc.vector.tensor_scalar(
                    out=bb[:], in0=bb[:], scalar1=-1.0, scalar2=None,
                    op0=OP.mult,
                )
                return rs, bb

            def h_update(hsrc, hdst, rs_h, bb_h):
                for s in range(NSL):
                    c0, w = slab_cols(s)
                    r = sp.tile([128, 512], TDT, tag="rh", bufs=2)
                    nc.scalar.activation(
                        out=r[:, :w], in_=agg[:, c0 : c0 + w],
                        func=AF.Relu, bias=bb_h[:], scale=rs_h[:],
                    )
                    rm = sp.tile([128, 512], TDT, tag="rm", bufs=2)
                    nc.vector.tensor_tensor(
                        out=rm[:, :w], in0=r[:, :w],
                        in1=nm_all[:, c0 : c0 + w], op=OP.mult,
                    )
                    nc.vector.tensor_tensor(
                        out=hdst[:, c0 : c0 + w], in0=hsrc[:, c0 : c0 + w],
                        in1=rm[:, :w], op=OP.add,
                    )

            if lvl >= 3:
                h_stats(0)
                pack0 = stp.tile([128, 4], F32, name="pack0")
                nc.vector.tensor_reduce(
                    out=pack0[:, 0:1], in_=hsum[0][:], op=OP.add, axis=AX.X
                )
                nc.vector.tensor_reduce(
                    out=pack0[:, 1:2], in_=hssq[0][:], op=OP.add, axis=AX.X
                )
                nc.vector.tensor_reduce(
                    out=pack0[:, 2:3], in_=ssum_e[:], op=OP.add, axis=AX.X
                )
                nc.vector.tensor_reduce(
                    out=pack0[:, 3:4], in_=ssq_e[:], op=OP.add, axis=AX.X
                )
                nc.sync.dma_start(out=cc_st_in[0][:], in_=pack0[:])
                nc.gpsimd.collective_compute(
                    "AllReduce", OP.add, replica_groups=rg,
                    ins=[cc_st_in[0][:]], outs=[cc_st_out[0][:]],
                )
                stt0 = stp.tile([128, 4], F32, name="stt0")
                nc.sync.dma_start(out=stt0[:], in_=cc_st_out[0][:])
                rs_h0, bb_h0 = bn_cols(stt0[:, 0:1], stt0[:, 1:2], N, "h0")
                rs_e, bb_e = bn_cols(stt0[:, 2:3], stt0[:, 3:4], E, "e0")

                h_update(hA, hB, rs_h0, bb_h0)
                boundary(1, hB)

                # ================= e update + layer-1 msg =================
                for gi, (t, j, S, c0) in enumerate(groups):
                    G = 128 // S
                    ec0 = c0 * 128
                    if gi % 4 == 0:
                        ew = min(2048, E_PAD - ec0)
                        zld = sp.tile([128, 2048], TDT, tag="zld", bufs=3)
                        nc.sync.dma_start(
                            out=zld[:, :ew], in_=z_buf[:, ec0 : ec0 + ew]
                        )
                        e0sl = sp.tile([F_E + 1, 2048], TDT, tag="e0sl", bufs=3)
                        nc.sync.dma_start(
                            out=e0sl[:, :ew], in_=e0T[:, ec0 : ec0 + ew]
                        )
                    o = (gi % 4) * 512
                    r0 = 4 * j * G
                    pe0 = ps.tile([128, 512], F32, tag="pA" if gi % 2 == 0 else "pD")
                    nc.tensor.matmul(
                        out=pe0[:], lhsT=ew9[:], rhs=e0sl[:, o : o + 512],
                        start=True, stop=True,
                    )
                    ret = sp.tile([128, 512], F32, tag="ret", bufs=2)
                    nc.scalar.activation(
                        out=ret[:], in_=zld[:, o : o + 512], func=AF.Relu,
                        bias=bb_e[:], scale=rs_e[:],
                    )
                    e1t = sp.tile([128, 512], F32, tag="e1t", bufs=2)
                    nc.vector.tensor_tensor(
                        out=e1t[:], in0=pe0[:], in1=ret[:], op=OP.add
                    )
                    wt = sp.tile([128, 512], F32, tag="wt", bufs=4)
                    nc.scalar.activation(out=wt[:], in_=e1t[:], func=AF.Sigmoid)
                    gt1 = sp.tile([128, 4, 128], F32, tag="gt1", bufs=6)
                    for k in range(4):
                        nc.gpsimd.indirect_dma_start(
                            out=gt1[:, k, :],
                            out_offset=None,
                            in_=cc_hout[1][:],
                            in_offset=IndirectOffsetOnAxis(
                                ap=srci[:, c0 + k : c0 + k + 1], axis=0
                            ),
                        )
                    pvc = ps.tile([128, 512], F32, tag="pC")
                    for k in range(4):
                        nc.tensor.matmul(
                            out=pvc[:, k * 128 : (k + 1) * 128],
                            lhsT=gt1[:, k, :], rhs=ident[:],
                            is_transpose=True, start=True, stop=True,
                            skip_group_check=True,
                        )
                    msg = sp.tile([128, 512], TDT, tag="msg", bufs=3)
                    nc.vector.tensor_tensor(
                        out=msg[:], in0=pvc[:], in1=wt[:], op=OP.mult
                    )
                    nc.vector.tensor_reduce(
                        out=agg[:, t * 128 + r0 : t * 128 + r0 + 4 * G],
                        in_=msg[:].rearrange("p (g s) -> p g s", s=S),
                        op=OP.max, axis=AX.X,
                    )

            if lvl >= 4:
                h_stats(1)
                pack1 = stp.tile([128, 2], F32, name="pack1")
                nc.vector.tensor_reduce(
                    out=pack1[:, 0:1], in_=hsum[1][:], op=OP.add, axis=AX.X
                )
                nc.vector.tensor_reduce(
                    out=pack1[:, 1:2], in_=hssq[1][:], op=OP.add, axis=AX.X
                )
                nc.sync.dma_start(out=cc_st_in[1][:], in_=pack1[:])
                nc.gpsimd.collective_compute(
                    "AllReduce", OP.add, replica_groups=rg,
                    ins=[cc_st_in[1][:]], outs=[cc_st_out[1][:]],
                )
                stt1 = stp.tile([128, 2], F32, name="stt1")
                nc.sync.dma_start(out=stt1[:], in_=cc_st_out[1][:])
                rs_h1, bb_h1 = bn_cols(stt1[:, 0:1], stt1[:, 1:2], N, "h1")
                h_update(hB, hC, rs_h1, bb_h1)

                for s in range(NSL):
                    c0, w = slab_cols(s)
                    scr = sp.tile([128, 512], TDT, tag="m01", bufs=2)
                    nc.vector.tensor_scalar(
                        out=scr[:, :w], in0=hC[:, c0 : c0 + w], scalar1=1.0,
                        scalar2=0.0, op0=OP.mult, op1=OP.add,
                        accum_out=moys[:, s : s + 1],
                    )
                boundary(2, hC)
                moyp = stp.tile([128, 1], F32, name="moyp")
                nc.vector.tensor_reduce(
                    out=moyp[:], in_=moys[:], op=OP.add, axis=AX.X
                )
                nc.sync.dma_start(out=cc_moy_in[:], in_=moyp[:])
                nc.gpsimd.collective_compute(
                    "AllReduce", OP.add, replica_groups=rg,
                    ins=[cc_moy_in[:]], outs=[cc_moy_out[:]],
                )
                moyc = stp.tile([128, 1], F32, name="moyc")
                nc.sync.dma_start(out=moyc[:], in_=cc_moy_out[:])
                nc.vector.tensor_scalar(
                    out=moyc[:], in0=moyc[:], scalar1=1.0 / N, scalar2=None,
                    op0=OP.mult,
                )
                pbase = ps.tile([128, 512], F32, tag="pD")
                nc.tensor.matmul(
                    out=pbase[:, 0:1], lhsT=W0a[:], rhs=moyc[:],
                    start=True, stop=True, skip_group_check=True,
                )
                base_col = stp.tile([128, 1], F32, name="base_col")
                nc.vector.tensor_tensor(
                    out=base_col[:], in0=pbase[:, 0:1], in1=W0bc[:], op=OP.add
                )

            # ================= readout =================
            if lvl < 5:
                ydummy = sp.tile([1, 512], F32, tag="ydummy", bufs=1)
                nc.gpsimd.memset(ydummy[:], 0.0)
                nc.sync.dma_start(out=y_out[0:1, 0:512], in_=ydummy[:])
            else:
                for gi, (t, j, S, c0) in enumerate(groups):
                    G = 128 // S
                    r0 = 4 * j * G
                    kront = sp.tile([128, 512], TDT, tag="kront", bufs=6)
                    nc.sync.dma_start(
                        out=kront[: 4 * G, :],
                        in_=kron_md[t * 128 + r0 : t * 128 + r0 + 4 * G, :],
                    )
                    band = sp.tile([128, 128], TDT, tag="band", bufs=6)
                    nc.sync.dma_start(
                        out=band[: 4 * G, :],
                        in_=hb_buf[t * 128 + r0 : t * 128 + r0 + 4 * G, :],
                    )
                    gt1 = sp.tile([128, 4, 128], F32, tag="gt1", bufs=6)
                    if ablate == "ro_nogather":
                        nc.vector.memset(gt1[:], 0.5)
                    else:
                        for k in range(4):
                            nc.gpsimd.indirect_dma_start(
                                out=gt1[:, k, :],
                                out_offset=None,
                                in_=cc_hout[2][:],
                                in_offset=IndirectOffsetOnAxis(
                                    ap=srci[:, c0 + k : c0 + k + 1], axis=0
                                ),
                            )
                    pk = ps.tile([128, 512], F32, tag="pB")
                    nc.tensor.matmul(
                        out=pk[:],
                        lhsT=band[: 4 * G, :],
                        rhs=kront[: 4 * G, :],
                        start=True, stop=False, skip_group_check=True,
                    )
                    for k in range(4):
                        nc.tensor.matmul(
                            out=pk[:, k * 128 : (k + 1) * 128],
                            lhsT=gt1[:, k, :], rhs=ident[:],
                            is_transpose=True, start=False,
                            stop=(k == 3), skip_group_check=True,
                        )
                    t1 = sp.tile([128, 512], RODT, tag="t1", bufs=5)
                    nc.scalar.activation(
                        out=t1[:], in_=pk[:], func=AF.Relu, bias=base_col[:]
                    )
                    if ablate == "ro_nomlp":
                        t3 = t1
                    else:
                        p2 = ps.tile([128, 512], F32, tag="pA")
                        nc.tensor.matmul(
                            out=p2[:], lhsT=Wk0[:], rhs=t1[:], start=True, stop=True
                        )
                        t2 = sp.tile([128, 512], RODT, tag="t2", bufs=5)
                        nc.vector.tensor_scalar(
                            out=t2[:], in0=p2[:], scalar1=Wkb0[:], scalar2=0.0,
                            op0=OP.add, op1=OP.max,
                        )
                        p3 = ps.tile([128, 512], F32, tag="pC")
                        nc.tensor.matmul(
                            out=p3[:], lhsT=Wk1[:], rhs=t2[:], start=True, stop=True
                        )
                        t3 = sp.tile([128, 512], RODT, tag="t3", bufs=5)
                        nc.vector.tensor_scalar(
                            out=t3[:], in0=p3[:], scalar1=Wkb1[:], scalar2=0.0,
                            op0=OP.add, op1=OP.max,
                        )
                    py = ps.tile([1, 512], F32, tag="pD")
                    nc.tensor.matmul(
                        out=py[:], lhsT=Wf[:], rhs=t3[:], start=True, stop=True,
                        skip_group_check=True,
                    )
                    yt = sp.tile([1, 512], F32, tag="yt", bufs=5)
                    nc.scalar.activation(
                        out=yt[:], in_=py[:], func=AF.Sigmoid, bias=wfb[:]
                    )
                    nc.sync.dma_start(
                        out=y_out[0:1, c0 * 128 : c0 * 128 + 512], in_=yt[:]
                    )

    nc.compile()
    return nc


# ---------------------------------------------------------------------------
# top level
# ---------------------------------------------------------------------------


def _bf16(x):
    import ml_dtypes

    return np.asarray(x, np.float32).astype(ml_dtypes.bfloat16)


def prepare(inputs):
    """Host prep: plan + per-core input maps. Returns (plan, in_maps, origids)."""
    h = np.asarray(inputs["h"], np.float32)
    e = np.asarray(inputs["e"], np.float32)
    src = np.asarray(inputs["src"]).astype(np.int64)
    dst = np.asarray(inputs["dst"]).astype(np.int64)
    N = h.shape[0]

    plan = _plan(src, dst, N)
    plan["F_N"] = h.shape[1]
    plan["F_E"] = e.shape[1]

    U = np.asarray(inputs["U"], np.float32)
    V = np.asarray(inputs["V"], np.float32)
    A = np.asarray(inputs["A"], np.float32)
    B = np.asarray(inputs["B"], np.float32)
    C = np.asarray(inputs["C"], np.float32)
    W0_w = np.asarray(inputs["W0_w"], np.float32)
    Wk_w = np.asarray(inputs["Wk_w"], np.float32)
    Wk_b = np.asarray(inputs["Wk_b"], np.float32)
    Wf_w = np.asarray(inputs["Wf_w"], np.float32)
    Wf_b = np.asarray(inputs["Wf_b"], np.float32)
    emb_e_w = np.asarray(inputs["emb_e_w"], np.float32)
    emb_e_b = np.asarray(inputs["emb_e_b"], np.float32)
    emb_n_w = np.asarray(inputs["emb_n_w"], np.float32)
    emb_n_b = np.asarray(inputs["emb_n_b"], np.float32)

    gne = np.zeros((1, 256), np.float32)
    gne[0, :128] = -1e30

    shared = dict(
        ident=np.eye(128, dtype=np.float32),
        gne=gne,
        ew9=_bf16(np.concatenate([emb_e_w, emb_e_b[None, :]], axis=0)),
        eA9=_bf16(
            np.concatenate([emb_e_w @ A[0], (emb_e_b @ A[0])[None, :]], axis=0)
        ),
        hw17=_bf16(np.concatenate([emb_n_w, emb_n_b[None, :]], axis=0)),
        B0=_bf16(B[0]),
        WnV18=_bf16(
            np.concatenate(
                [
                    emb_n_w @ V[0],
                    (emb_n_b @ V[0])[None, :],
                    np.full((1, 128), -1e30, np.float32),
                ],
                axis=0,
            )
        ),
        WnC18=_bf16(
            np.concatenate(
                [
                    emb_n_w @ C[0],
                    (emb_n_b @ C[0])[None, :],
                    np.zeros((1, 128), np.float32),
                ],
                axis=0,
            )
        ),
        V1=_bf16(V[1]),
        U0=_bf16(U[0]),
        U1=_bf16(U[1]),
        WBC=_bf16(np.concatenate([W0_w[128:256], W0_w[256:384]], axis=1)),
        W0a=np.ascontiguousarray(W0_w[:128]),
        W0b_col=np.asarray(inputs["W0_b"], np.float32).reshape(128, 1),
        Wk0=_bf16(Wk_w[0]),
        Wk1=_bf16(Wk_w[1]),
        Wkb0=Wk_b[0].reshape(128, 1).astype(np.float32),
        Wkb1=Wk_b[1].reshape(128, 1).astype(np.float32),
        Wf=_bf16(Wf_w.reshape(128, 1)),
        wfb=np.full((1, 1), float(Wf_b), np.float32),
    )

    in_maps = []
    origids = []
    for d in range(NC):
        pc = _per_core_arrays(plan, d, h, e, _bf16)
        origids.append(pc.pop("origid"))
        m = dict(pc)
        m.update(shared)
        in_maps.append(m)
    return plan, in_maps, origids


def unshard(plan, origids, results, E):
    out = np.zeros(E, np.float32)
    for d in range(NC):
        y = np.asarray(results[d]["y"]).reshape(-1)
        oid = origids[d]
        valid = oid >= 0
        out[oid[valid]] = y[valid]
    return out


def kernel(**inputs):
    import sys

    if "/opt/trn_rl_repo" not in sys.path:
        sys.path.insert(0, "/opt/trn_rl_repo")
    from concourse.bass_utils import run_bass_kernel_spmd

    plan, in_maps, origids = prepare(inputs)
    nc = _build_program(plan)
    res = run_bass_kernel_spmd(nc, in_maps, list(range(NC)))
    return unshard(plan, origids, res.results, plan["E"])
